# revision 27
# baseline (speedup 1.0000x reference)
"""Linear (kernel-feature) attention for Trainium2, sharded over 8 NeuronCores.

Problem: B=4, H=16, S=4096, D=64 fp32.
    phi(x) = elu(x) + 1 = exp(min(x,0)) + relu(x)
    kv   = phi_k_masked^T @ V          [d, v]
    k1   = phi_k^T @ mask              [d]
    out  = (phi_q @ kv) / (phi_q @ k1 + eps)

Sharding: 64 (b,h) slices -> 8 per core, processed as 4 pairs.
All device data is bf16 (tolerance 2e-2; measured end-to-end error of this
pipeline is ~3.3e-3), halving the HBM roofline to ~16 MB/core and doubling
PE stream rate + enabling FWL weight loads.

Host-side layout (part of sharding, costs no HW time). One fused input
tensor per pair for a single ~3.1 MB DMA (best HBM efficiency):
  inp[pair] = [128, 12352] bf16, cols:
    [0, 4096)      kp: per 128-row n-tile t, cols t*128+{0:64,64:128} =
                   slice {a,b}'s K tile [n-part, d]
    [4096, 8256)   vx: per tile, cols 4096+t*130+{0:65,65:130} = slice
                   {a,b}'s [V | mask] (mask col fuses k1 into the kv
                   matmul; mask also folded into V on host if not all-1)
    [8256, 12352)  qT: partitions 0:64 = slice a's (q/8)^T [d, n], 64:128 =
                   b's (q pre-scaled by 1/8 on host - exact in bf16 - so
                   phi_q has the identical form to phi_k on device)
  outc[pair] = [128, 2*FREE] bf16, natural [n-part, t, d] per slice.

Device pipeline (v3 - DMA-bound design, every compute engine < DMA):
  ALL input DMAs are issued up-front (ipool holds all 4 pairs) so the 16
  HW DMA engines stream the 12.6 MB input contiguously; the v1 design
  issued pair p's input behind pair p-1's OUTPUT dma_starts on the sync
  queue, which head-of-line-blocked the input stream on compute (~26us of
  mid-run DMA-idle gaps).
  Work is spread across ALL FIVE engines to keep each under the DMA time
  (measured DVE rates: tensor_scalar 4x on bf16 SBUF, tensor_tensor 2x
  max, any-PSUM operand 1x, scalar_tensor_tensor 1x-only):
    phi_k: xm=min(k,0) [DVE 4x], ek=exp(xm) [ACT], rk=max(k,0) [DVE 4x];
    M1 runs TWO weight chains (rk, ek) into one psum group - the phi_k
    add happens on the PE via accumulation, phi_k never materialized.
    phi_q: xq/eq/rq same, then pq = rq + eq [DVE tensor_tensor 2x]
    (M2 needs materialized full-width weights).
  M1: 64-matmul chain (rk|ek) @ [V|m] -> kv_ext psum [128,130]; valid
  quadrants [0:64,0:65]=slice a, [64:128,65:130]=slice b; garbage
  quadrants zeroed in the bf16 copy so M2 can run full-K (enables FWL).
  M2: per slice, one matmul per n-tile with fused [kv|k1] rhs (N=65) into
  7-tile psum groups at 512-col offsets of a 3-bank + 2-bank psum tile
  pair; DVE reciprocal batched over the strided nrm columns (eps dropped:
  nrm ~ 1e5 >> 1e-6). Evac (psum -> out_sb bf16, x recip broadcast) is
  split per slice to offload the v2 DVE bottleneck: slice a goes ACT
  (4D-AP psum->sbuf bf16 copy, dropping the nrm cols) + one batched
  GPSIMD broadcast multiply (GPSIMD has no PSUM port, hence the copy);
  slice b is the direct DVE broadcast multiply. Per-slice output DMA.
  Two-stage software pipeline (front(p+1) emitted before back(p)) so no
  engine FIFO head-of-line-blocks on a cross-engine dependency.
"""

import sys

sys.path.insert(0, "/opt/trn_rl_repo")

import numpy as np
import ml_dtypes

B, H, S, D = 4, 16, 4096, 64
N_CORES = 8
SL = (B * H) // N_CORES  # slices per core = 8
PAIRS = SL // 2  # 4
NT = S // 128  # 32 n-tiles per slice
FREE = NT * D  # 2048 free cols per slice of output
KW = NT * 128  # 4096: packed-K region width
VW = NT * 130  # 4160: packed-[V|m] region width
INW = KW + VW + S  # 12352: fused input width
BF16 = ml_dtypes.bfloat16

_programs: dict = {}


def _build_program():
    from contextlib import ExitStack

    import concourse.bacc as bacc
    import concourse.tile as tile
    from concourse import mybir

    f32 = mybir.dt.float32
    bf16 = mybir.dt.bfloat16
    Alu = mybir.AluOpType
    Act = mybir.ActivationFunctionType

    nc = bacc.Bacc("TRN2", target_bir_lowering=False, debug=False)
    inp = nc.dram_tensor("inp", [PAIRS, 128, INW], bf16, kind="ExternalInput").ap()
    outc = nc.dram_tensor(
        "outc", [PAIRS, 128, 2 * FREE], bf16, kind="ExternalOutput"
    ).ap()

    # M2 tile groups: psum bank holds 7 tiles of 65 cols (455 <= 512)
    GROUPS = [(0, 7), (7, 7), (14, 7), (21, 7), (28, 4)]

    with tile.TileContext(nc) as tc, ExitStack() as ctx:
        ipool = ctx.enter_context(tc.tile_pool(name="ipool", bufs=4))
        # 4-deep rotation on the M1 chain-weight tiles (xm/ek): reuse
        # distance 2 pairs, so a pair's phi writes never FIFO-stall on the
        # previous pair's M1 chain still reading the recycled buffer.
        tmp = ctx.enter_context(tc.tile_pool(name="tmp", bufs=4))
        pqp = ctx.enter_context(tc.tile_pool(name="pqp", bufs=2))
        kvp = ctx.enter_context(tc.tile_pool(name="kvp", bufs=2))
        nrmp = ctx.enter_context(tc.tile_pool(name="nrmp", bufs=4))
        stp = ctx.enter_context(tc.tile_pool(name="stp", bufs=2))
        outp = ctx.enter_context(tc.tile_pool(name="outp", bufs=2))
        ps_kv = ctx.enter_context(tc.tile_pool(name="ps_kv", bufs=2, space="PSUM"))
        # M2 psum: 3-bank + 2-bank tiles (groups at 512-col offsets) so the
        # nrm reciprocals and the evac multiply batch across groups.
        ps_a = ctx.enter_context(tc.tile_pool(name="ps_a", bufs=1, space="PSUM"))
        ps_b = ctx.enter_context(tc.tile_pool(name="ps_b", bufs=1, space="PSUM"))

        # All input DMAs up-front: the input stream must never stall on
        # compute (it is the bottleneck; every engine has ~2x slack).
        its = []
        for pair in range(PAIRS):
            it = ipool.tile([128, INW], bf16)
            # 3-way split (range deps): phi_k needs only k, M1 adds v,
            # phi_q waits for q - each stage starts as early as possible.
            nc.sync.dma_start(out=it[:, 0:KW], in_=inp[pair][:, 0:KW])
            nc.sync.dma_start(out=it[:, KW : KW + VW], in_=inp[pair][:, KW : KW + VW])
            nc.sync.dma_start(out=it[:, KW + VW : INW], in_=inp[pair][:, KW + VW : INW])
            its.append(it)

        def fa(pair):
            """phi pieces: DVE xm/rk/xq/pq-relu, ACT ek/eq (the pacing ops).
            2-chain M1 everywhere: PE must stay light because M2 slice-b
            inevitably stalls ~2us on slice-a's psum readers (psum bufs=1);
            only a slack PE absorbs that."""
            it = its[pair]
            kt = it[:, 0:KW]
            qt = it[:, KW + VW : INW]

            # phi_k = exp(min(k,0)) + relu(k): the adds happen on the PE
            # via psum accumulation; phi_k never materialized.
            # Pairs 0-2: 3 chains (raw k [zero engine deps], xm = -min(k,0)
            # [one fused DVE op], exp(-xm) [ACT]) - keeps DVE (the #2
            # engine) to ONE op per K tensor; PE has the slack.
            # Last pair: 2 chains (rk = relu(k) [DVE], exp) - halves the
            # V-gated matmul work on the drain critical path.
            last = pair == PAIRS - 1
            rk = None
            if last:
                xm = tmp.tile([128, S], bf16, tag="x")
                nc.vector.tensor_scalar_min(xm, kt, 0.0)
                ek = tmp.tile([128, S], bf16, tag="e")
                nc.scalar.activation(ek, xm, Act.Exp)
                rk = tmp.tile([128, S], bf16, tag="x")
                nc.vector.tensor_scalar_max(rk, kt, 0.0)
            else:
                xm = tmp.tile([128, S], bf16, tag="x")
                nc.vector.tensor_scalar(xm, kt, 0.0, -1.0, Alu.min, Alu.mult)
                ek = tmp.tile([128, S], bf16, tag="e")
                nc.scalar.activation(ek, xm, Act.Exp, scale=-1.0)

            # phi_q = exp(min(qs,0)) + relu(qs), qs = q/8 (host-scaled).
            # relu part lands directly in the pq tile; m() adds eq on top.
            # (GPSIMD can't take this op: walrus rejects Pool-engine
            # TENSOR_TENSOR with a max ALU, and Pool TENSOR_SCALAR runs at
            # ~16ns/elem - only its mult TENSOR_TENSOR is usable.)
            xq = tmp.tile([128, S], bf16, tag="x")
            nc.vector.tensor_scalar_min(xq, qt, 0.0)
            pq = pqp.tile([128, S], bf16)
            nc.vector.tensor_scalar_max(pq, qt, 0.0)
            eq = tmp.tile([128, S], bf16, tag="e")
            nc.scalar.activation(eq, xq, Act.Exp)
            return {"rk": rk, "xm": xm, "ek": ek, "eq": eq, "pq": pq}

        def fb(pair, st):
            """M1 accumulation chains (PE only)."""
            it = its[pair]
            vt = it[:, KW : KW + VW]
            it2 = its[pair]
            srcs = (
                (st["rk"], st["ek"])
                if st["rk"] is not None
                else (it2[:, 0:KW], st["xm"], st["ek"])
            )
            kv_ps = ps_kv.tile([128, 130], f32)
            for src in srcs:
                for t in range(NT):
                    nc.tensor.matmul(
                        kv_ps,
                        src[:, t * 128 : (t + 1) * 128],
                        vt[:, t * 130 : (t + 1) * 130],
                        start=(src is srcs[0] and t == 0),
                        stop=(src is st["ek"] and t == NT - 1),
                    )
            st["kv_ps"] = kv_ps

        def m(pair, st):
            """kv evac to bf16 + pq assembly. Emitted one pair behind fa/fb
            so none of these ops FIFO-stall their engine on M1/eq."""
            kv_ps = st["kv_ps"]
            # zero the garbage quadrants so M2 can use full-K (128) weights:
            # the wrong slice's pq half multiplies zeros. Full-width weights
            # also enable FWL.
            kv_sb = kvp.tile([128, 130], bf16)
            nc.vector.memset(kv_sb, 0.0)
            nc.scalar.copy(kv_sb[0:64, 0:65], kv_ps[0:64, 0:65])
            nc.scalar.copy(kv_sb[64:128, 65:130], kv_ps[64:128, 65:130])
            nc.vector.tensor_tensor(st["pq"], st["pq"], st["eq"], Alu.add)
            st["kv_sb"] = kv_sb

        def b_pe(pair, st):
            """M2 chains + the psum-side DVE work (recips, slice-b evac)."""
            kv_sb, pq = st["kv_sb"], st["pq"]
            out_sb = outp.tile([128, 2 * FREE], bf16)
            st["out_sb"] = out_sb
            st["ps"] = []
            for rr in range(2):
                # [kv | k1] for this slice; other slice's quadrant is zero,
                # so full-K (128) pq weights are safe and FWL-eligible
                rhs = kv_sb[:, 0:65] if rr == 0 else kv_sb[:, 65:130]
                poa = ps_a.tile([128, 1536], f32)  # groups 0-2, 512-col offsets
                pob = ps_b.tile([128, 1024], f32)  # groups 3-4
                for gi, (g0, gs) in enumerate(GROUPS):
                    po = poa if gi < 3 else pob
                    off = 512 * (gi if gi < 3 else gi - 3)
                    for i in range(gs):
                        t = g0 + i
                        nc.tensor.matmul(
                            po[:, off + i * 65 : off + (i + 1) * 65],
                            pq[:, t * 128 : (t + 1) * 128],
                            rhs,
                            start=(i == 0),
                            stop=(i == gs - 1),
                            skip_group_check=True,
                        )
                pa4 = (
                    poa.rearrange("p (g x) -> p g x", x=512)[:, :, 0:455]
                    .rearrange("p g (i c) -> p g i c", c=65)
                )
                pb4 = (
                    pob.rearrange("p (g x) -> p g x", x=512)[:, :, 0:455]
                    .rearrange("p g (i c) -> p g i c", c=65)
                )
                if rr == 0:
                    # slice a: batched recips now; psum->sbuf copy + the
                    # broadcast multiply happen in b_evac (ACT + GPSIMD).
                    ns = nrmp.tile([128, 32], f32, tag="nsg")
                    nc.vector.reciprocal(
                        ns[:, 0:21].rearrange("p (g i o) -> p g i o", g=3, o=1),
                        pa4[:, :, :, 64:65],
                    )
                    nc.vector.reciprocal(
                        ns[:, 21:28].rearrange("p (i o) -> p i o", o=1),
                        pb4[:, 0, 0:7, 64:65],
                    )
                    nc.vector.reciprocal(
                        ns[:, 28:32].rearrange("p (i o) -> p i o", o=1),
                        pb4[:, 1, 0:4, 64:65],
                    )
                    st["ns"] = ns
                    st["pa4"], st["pb4"] = pa4, pb4
                    # slice a psum stays live until b_evac's ACT copies; M2b
                    # uses the OTHER psum pool generation (bufs=2).
                else:
                    # slice b: direct DVE broadcast multiplies from psum
                    base = FREE
                    nsa = nrmp.tile([128, 21], f32, tag="nsa")
                    nsa3 = nsa.rearrange("p (g i) -> p g i", g=3)
                    nc.vector.reciprocal(
                        nsa.rearrange("p (g i o) -> p g i o", g=3, o=1),
                        pa4[:, :, :, 64:65],
                    )
                    nc.vector.tensor_tensor(
                        out_sb[:, base : base + 1344].rearrange(
                            "p (g i c) -> p g i c", g=3, c=64
                        ),
                        pa4[:, :, :, 0:64],
                        nsa3.broadcast_to([128, 3, 7, 64]),
                        Alu.mult,
                    )
                    nsb = nrmp.tile([128, 14], f32, tag="nsb")
                    nsb3 = nsb.rearrange("p (g i) -> p g i", g=2)
                    for bi, (g0, gs) in enumerate(GROUPS[3:]):
                        nc.vector.reciprocal(
                            nsb[:, bi * 7 : bi * 7 + gs].rearrange(
                                "p (i o) -> p i o", o=1
                            ),
                            pb4[:, bi, 0:gs, 64:65],
                        )
                    for bi, (g0, gs) in enumerate(GROUPS[3:]):
                        nc.vector.tensor_tensor(
                            out_sb[
                                :, base + g0 * 64 : base + (g0 + gs) * 64
                            ].rearrange("p (i c) -> p i c", c=64),
                            pb4[:, bi, 0:gs, 0:64],
                            nsb3[:, bi, 0:gs].broadcast_to([128, gs, 64]),
                            Alu.mult,
                        )

        def b_evac(pair, st):
            """Slice-a evac (ACT psum->sbuf copies + one GPSIMD broadcast
            multiply; GPSIMD has no PSUM port) + the pair's single output
            DMA, issued from the SCALAR HWDGE ring - critically NOT the
            sync ring, where its descriptors would sit behind the entire
            remaining INPUT stream (and not GPSIMD SWDGE, which moved the
            4.2 MB of outputs at only ~100 GB/s). Emitted two pairs behind
            fa so the ACT copies never FIFO-stall the pacing ek/eq ops."""
            out_sb, ns = st["out_sb"], st["ns"]
            pa4, pb4 = st["pa4"], st["pb4"]
            stage = stp.tile([128, FREE], bf16)
            nc.scalar.copy(
                stage[:, 0:1344].rearrange("p (g i c) -> p g i c", g=3, c=64),
                pa4[:, :, :, 0:64],
            )
            nc.scalar.copy(
                stage[:, 1344:1792].rearrange("p (i c) -> p i c", c=64),
                pb4[:, 0, 0:7, 0:64],
            )
            nc.scalar.copy(
                stage[:, 1792:2048].rearrange("p (i c) -> p i c", c=64),
                pb4[:, 1, 0:4, 0:64],
            )
            nc.gpsimd.tensor_tensor(
                out_sb[:, 0:FREE].rearrange("p (i c) -> p i c", c=64),
                stage.rearrange("p (i c) -> p i c", c=64),
                ns.rearrange("p i -> p i ()").broadcast_to([128, 32, 64]),
                Alu.mult,
            )
            nc.scalar.dma_start(out=outc[pair][:, :], in_=out_sb[:, :])

        # Software pipeline, staggered per engine so every FIFO always has
        # ready work ahead of any dependency-blocked instruction.
        st = {}
        st[0] = fa(0)
        fb(0, st[0])
        st[1] = fa(1)
        fb(1, st[1])
        m(0, st[0])
        b_pe(0, st[0])
        b_evac(0, st[0])
        st[2] = fa(2)
        fb(2, st[2])
        m(1, st[1])
        b_pe(1, st[1])
        b_evac(1, st[1])
        st[3] = fa(3)
        fb(3, st[3])
        m(2, st[2])
        b_pe(2, st[2])
        b_evac(2, st[2])
        m(3, st[3])
        b_pe(3, st[3])
        b_evac(3, st[3])

    nc.compile()
    return nc


def _get_program():
    if "p" not in _programs:
        _programs["p"] = _build_program()
    return _programs["p"]


def _pack_inputs(query, key, value, attention_mask):
    """Shard + lay out inputs for the 8 cores (all bf16, fused per pair)."""
    q4 = np.asarray(query, dtype=np.float32).reshape(B * H, S, D)
    k4 = np.asarray(key, dtype=np.float32).reshape(B * H, S, D)
    v4 = np.asarray(value, dtype=np.float32).reshape(B * H, S, D)
    am = np.asarray(attention_mask, dtype=np.float32)

    inp = np.empty((N_CORES, PAIRS, 128, INW), dtype=BF16)
    # kp: [g, n, d] -> [core, pair, p, t*128 + s*64 + d]
    k6 = k4.reshape(N_CORES, PAIRS, 2, NT, 128, D)
    inp[:, :, :, 0:KW] = (
        k6.transpose(0, 1, 4, 3, 2, 5).reshape(N_CORES, PAIRS, 128, KW).astype(BF16)
    )
    # vx: [V*mask | mask] -> [core, pair, p, KW + t*130 + s*65 + c]
    mrow = np.repeat(am, H, axis=0).reshape(B * H, S, 1)  # [g, n, 1]
    if np.all(am == 1.0):
        vext = np.concatenate([v4, mrow], axis=-1)
    else:
        vext = np.concatenate([v4 * mrow, mrow], axis=-1)
    v6 = vext.reshape(N_CORES, PAIRS, 2, NT, 128, D + 1)
    inp[:, :, :, KW : KW + VW] = (
        v6.transpose(0, 1, 4, 3, 2, 5).reshape(N_CORES, PAIRS, 128, VW).astype(BF16)
    )
    # qT: [g, d, n] -> [core, pair, 2*64 d, n], pre-scaled by 1/8 (exact)
    inp[:, :, :, KW + VW : INW] = (
        np.ascontiguousarray((q4 * 0.125).transpose(0, 2, 1))
        .reshape(N_CORES, PAIRS, 2 * D, S)
        .astype(BF16)
    )
    return [{"inp": inp[c]} for c in range(N_CORES)]


def _unpack_output(results):
    outs = np.stack([r["outc"] for r in results])  # [cores, PAIRS, 128, 2*FREE]
    outs = outs.astype(np.float32).reshape(N_CORES, PAIRS, 128, 2, NT, D)
    outs = outs.transpose(0, 1, 3, 4, 2, 5)  # [cores, pair, s, t, p, d]
    return np.ascontiguousarray(outs).reshape(B, H, S, D)


def kernel(query, key, value, attention_mask):
    from concourse.bass_utils import run_bass_kernel_spmd

    in_maps = _pack_inputs(query, key, value, attention_mask)
    nc = _get_program()
    res = run_bass_kernel_spmd(nc, in_maps, core_ids=list(range(N_CORES)))
    return _unpack_output(res.results)


# revision 28
# speedup vs baseline: 1.0396x; 1.0396x over previous
"""Linear (kernel-feature) attention for Trainium2, sharded over 8 NeuronCores.

Problem: B=4, H=16, S=4096, D=64 fp32.
    phi(x) = elu(x) + 1 = exp(min(x,0)) + relu(x)
    kv   = phi_k_masked^T @ V          [d, v]
    k1   = phi_k^T @ mask              [d]
    out  = (phi_q @ kv) / (phi_q @ k1 + eps)

Sharding: 64 (b,h) slices -> 8 per core, processed as 4 pairs.
All device data is bf16 (tolerance 2e-2; measured end-to-end error of this
pipeline is ~3.3e-3), halving the HBM roofline to ~16 MB/core and doubling
PE stream rate + enabling FWL weight loads.

Host-side layout (part of sharding, costs no HW time). One fused input
tensor per pair for a single ~3.1 MB DMA (best HBM efficiency):
  inp[pair] = [128, 12352] bf16, cols:
    [0, 4096)      kp: per 128-row n-tile t, cols t*128+{0:64,64:128} =
                   slice {a,b}'s K tile [n-part, d]
    [4096, 8256)   vx: per tile, cols 4096+t*130+{0:65,65:130} = slice
                   {a,b}'s [V | mask] (mask col fuses k1 into the kv
                   matmul; mask also folded into V on host if not all-1)
    [8256, 12352)  qT: partitions 0:64 = slice a's (q/8)^T [d, n], 64:128 =
                   b's (q pre-scaled by 1/8 on host - exact in bf16 - so
                   phi_q has the identical form to phi_k on device)
  outc[pair] = [128, 2*FREE] bf16, natural [n-part, t, d] per slice.

Device pipeline (v9 - measured-balance design; HW 77.5us vs v1's 94.7us
on the same container; all engine numbers below are trace-measured):
  ALL 12 input DMAs are issued up-front on the SYNC HWDGE ring (ipool
  holds all 4 pairs) - the 16 SDMA engines then stream the 12.65 MB
  input gap-free at ~380 GB/s. Outputs are ONE DMA per pair issued from
  the SCALAR HWDGE ring: a separate descriptor ring is essential -
  output descriptors on the sync ring sit behind the entire remaining
  input stream (serializing out after in, v1's main stall), and GPSIMD
  SWDGE moved the outputs at only ~100 GB/s.
  Work split (measured op rates: DVE tensor_scalar 4x on bf16 SBUF
  ~1.6us, tensor_tensor 2x ~2.2us, any-PSUM operand 1x; ACT activation
  (224+FD)/1.2GHz ~3.6us; GPSIMD tensor_tensor-mult ~1.8ns/elem but
  tensor_scalar ~16ns/elem and TT-max rejected by walrus codegen):
    phi_k = exp(min(k,0)) + relu(k): xm/rk [DVE], ek [ACT]; the add
    happens on the PE - M1 runs TWO weight chains (rk, ek) into one psum
    accumulation group, so phi_k is never materialized. (The v1 3-chain
    form (k, -min, exp) trades ~1.6us DVE for ~2.5us PE per pair; with
    this schedule PE slack is needed for the M2 psum-recycle stall, and
    the 2-chain form measured faster end-to-end.)
    phi_q: xq=min [DVE], eq=exp(xq) [ACT], relu lands in the pq tile
    [DVE] and pq += eq in-place [DVE TT] one beat later, so the eq-gated
    add never blocks the DVE FIFO while recips/evacs are ready.
  M1: 64-matmul chain (rk|ek) @ [V|m] -> kv_ext psum [128,130]; valid
  quadrants [0:64,0:65]=slice a, [64:128,65:130]=slice b; garbage
  quadrants zeroed in the bf16 copy so M2 can run full-K (enables FWL).
  M2: per slice, one matmul per n-tile with fused [kv|k1] rhs (N=65) into
  7-tile psum groups at 512-col offsets of a 3-bank + 2-bank psum tile
  pair; DVE reciprocal batched over the strided nrm columns (eps dropped:
  nrm ~ 1e5 >> 1e-6). Evac (psum -> out_sb bf16, x recip broadcast) is
  split per slice to keep DVE under the roofline: slice a goes ACT
  (4D-AP psum->sbuf bf16 copies, nrm cols dropped) + one batched GPSIMD
  broadcast multiply (GPSIMD has no PSUM port, hence the copies); slice
  b is the direct DVE broadcast multiply. The slice-a psum readers run
  in the same beat as M2a: with psum bufs=1, M2b's bank reuse waits on
  them, so deferring them (tried) stalls the PE FIFO pipeline-wide.
  Emission is a staggered software pipeline (fa/fb -> m -> b_pe/b_evac,
  one-beat offsets) so every engine FIFO always has ready work ahead of
  any dependency-blocked instruction; the last pair runs a 2-chain M1
  (its V region lands last, and every M1 matmul streams V) to shorten
  the drain.
"""

import sys

sys.path.insert(0, "/opt/trn_rl_repo")

import numpy as np
import ml_dtypes

B, H, S, D = 4, 16, 4096, 64
N_CORES = 8
SL = (B * H) // N_CORES  # slices per core = 8
PAIRS = SL // 2  # 4
NT = S // 128  # 32 n-tiles per slice
FREE = NT * D  # 2048 free cols per slice of output
KW = NT * 128  # 4096: packed-K region width
VW = NT * 130  # 4160: packed-[V|m] region width
INW = KW + VW + S  # 12352: fused input width
BF16 = ml_dtypes.bfloat16

_programs: dict = {}


def _build_program():
    from contextlib import ExitStack

    import concourse.bacc as bacc
    import concourse.tile as tile
    from concourse import mybir

    f32 = mybir.dt.float32
    bf16 = mybir.dt.bfloat16
    Alu = mybir.AluOpType
    Act = mybir.ActivationFunctionType

    nc = bacc.Bacc("TRN2", target_bir_lowering=False, debug=False)
    inp = nc.dram_tensor("inp", [PAIRS, 128, INW], bf16, kind="ExternalInput").ap()
    outc = nc.dram_tensor(
        "outc", [PAIRS, 128, 2 * FREE], bf16, kind="ExternalOutput"
    ).ap()

    # M2 tile groups: psum bank holds 7 tiles of 65 cols (455 <= 512)
    GROUPS = [(0, 7), (7, 7), (14, 7), (21, 7), (28, 4)]

    with tile.TileContext(nc) as tc, ExitStack() as ctx:
        ipool = ctx.enter_context(tc.tile_pool(name="ipool", bufs=4))
        # 4-deep rotation on the M1 chain-weight tiles (xm/ek): reuse
        # distance 2 pairs, so a pair's phi writes never FIFO-stall on the
        # previous pair's M1 chain still reading the recycled buffer.
        tmp = ctx.enter_context(tc.tile_pool(name="tmp", bufs=4))
        pqp = ctx.enter_context(tc.tile_pool(name="pqp", bufs=2))
        kvp = ctx.enter_context(tc.tile_pool(name="kvp", bufs=2))
        nrmp = ctx.enter_context(tc.tile_pool(name="nrmp", bufs=4))
        stp = ctx.enter_context(tc.tile_pool(name="stp", bufs=2))
        outp = ctx.enter_context(tc.tile_pool(name="outp", bufs=2))
        ps_kv = ctx.enter_context(tc.tile_pool(name="ps_kv", bufs=2, space="PSUM"))
        # M2 psum: 3-bank + 2-bank tiles (groups at 512-col offsets) so the
        # nrm reciprocals and the evac multiply batch across groups.
        ps_a = ctx.enter_context(tc.tile_pool(name="ps_a", bufs=1, space="PSUM"))
        ps_b = ctx.enter_context(tc.tile_pool(name="ps_b", bufs=1, space="PSUM"))

        # All input DMAs up-front: the input stream must never stall on
        # compute (it is the bottleneck; every engine has ~2x slack).
        its = []
        for pair in range(PAIRS):
            it = ipool.tile([128, INW], bf16)
            # 3-way split (range deps): phi_k needs only k, M1 adds v,
            # phi_q waits for q - each stage starts as early as possible.
            nc.sync.dma_start(out=it[:, 0:KW], in_=inp[pair][:, 0:KW])
            nc.sync.dma_start(out=it[:, KW : KW + VW], in_=inp[pair][:, KW : KW + VW])
            nc.sync.dma_start(out=it[:, KW + VW : INW], in_=inp[pair][:, KW + VW : INW])
            its.append(it)

        def fa(pair):
            """phi pieces: DVE xm/rk/xq/pq-relu, ACT ek/eq (the pacing ops).
            2-chain M1 everywhere: PE must stay light because M2 slice-b
            inevitably stalls ~2us on slice-a's psum readers (psum bufs=1);
            only a slack PE absorbs that."""
            it = its[pair]
            kt = it[:, 0:KW]
            qt = it[:, KW + VW : INW]

            # phi_k = exp(min(k,0)) + relu(k): the add happens on the PE
            # via a 2-chain psum accumulation; phi_k never materialized.
            xm = tmp.tile([128, S], bf16, tag="x")
            nc.vector.tensor_scalar_min(xm, kt, 0.0)
            ek = tmp.tile([128, S], bf16, tag="e")
            nc.scalar.activation(ek, xm, Act.Exp)
            rk = tmp.tile([128, S], bf16, tag="x")
            nc.vector.tensor_scalar_max(rk, kt, 0.0)

            # phi_q = exp(min(qs,0)) + relu(qs), qs = q/8 (host-scaled).
            # relu part lands directly in the pq tile; m() adds eq on top.
            # (GPSIMD can't take this op: walrus rejects Pool-engine
            # TENSOR_TENSOR with a max ALU, and Pool TENSOR_SCALAR runs at
            # ~16ns/elem - only its mult TENSOR_TENSOR is usable.)
            xq = tmp.tile([128, S], bf16, tag="x")
            nc.vector.tensor_scalar_min(xq, qt, 0.0)
            pq = pqp.tile([128, S], bf16)
            nc.vector.tensor_scalar_max(pq, qt, 0.0)
            eq = tmp.tile([128, S], bf16, tag="e")
            nc.scalar.activation(eq, xq, Act.Exp)
            return {"rk": rk, "ek": ek, "eq": eq, "pq": pq}

        def fb(pair, st):
            """M1 accumulation chains (PE only)."""
            it = its[pair]
            vt = it[:, KW : KW + VW]
            srcs = (st["rk"], st["ek"])
            kv_ps = ps_kv.tile([128, 130], f32)
            for src in srcs:
                for t in range(NT):
                    nc.tensor.matmul(
                        kv_ps,
                        src[:, t * 128 : (t + 1) * 128],
                        vt[:, t * 130 : (t + 1) * 130],
                        start=(src is srcs[0] and t == 0),
                        stop=(src is st["ek"] and t == NT - 1),
                    )
            st["kv_ps"] = kv_ps

        def m(pair, st):
            """kv evac to bf16 + pq assembly. Emitted one pair behind fa/fb
            so none of these ops FIFO-stall their engine on M1/eq."""
            kv_ps = st["kv_ps"]
            # zero the garbage quadrants so M2 can use full-K (128) weights:
            # the wrong slice's pq half multiplies zeros. Full-width weights
            # also enable FWL.
            kv_sb = kvp.tile([128, 130], bf16)
            nc.vector.memset(kv_sb, 0.0)
            nc.scalar.copy(kv_sb[0:64, 0:65], kv_ps[0:64, 0:65])
            nc.scalar.copy(kv_sb[64:128, 65:130], kv_ps[64:128, 65:130])
            nc.vector.tensor_tensor(st["pq"], st["pq"], st["eq"], Alu.add)
            st["kv_sb"] = kv_sb

        def b_pe(pair, st):
            """M2 chains + the psum-side DVE work (recips, slice-b evac)."""
            kv_sb, pq = st["kv_sb"], st["pq"]
            out_sb = outp.tile([128, 2 * FREE], bf16)
            st["out_sb"] = out_sb
            st["ps"] = []
            for rr in range(2):
                # [kv | k1] for this slice; other slice's quadrant is zero,
                # so full-K (128) pq weights are safe and FWL-eligible
                rhs = kv_sb[:, 0:65] if rr == 0 else kv_sb[:, 65:130]
                poa = ps_a.tile([128, 1536], f32)  # groups 0-2, 512-col offsets
                pob = ps_b.tile([128, 1024], f32)  # groups 3-4
                for gi, (g0, gs) in enumerate(GROUPS):
                    po = poa if gi < 3 else pob
                    off = 512 * (gi if gi < 3 else gi - 3)
                    for i in range(gs):
                        t = g0 + i
                        nc.tensor.matmul(
                            po[:, off + i * 65 : off + (i + 1) * 65],
                            pq[:, t * 128 : (t + 1) * 128],
                            rhs,
                            start=(i == 0),
                            stop=(i == gs - 1),
                            skip_group_check=True,
                        )
                pa4 = (
                    poa.rearrange("p (g x) -> p g x", x=512)[:, :, 0:455]
                    .rearrange("p g (i c) -> p g i c", c=65)
                )
                pb4 = (
                    pob.rearrange("p (g x) -> p g x", x=512)[:, :, 0:455]
                    .rearrange("p g (i c) -> p g i c", c=65)
                )
                if rr == 0:
                    # slice a: batched recips now; psum->sbuf copy + the
                    # broadcast multiply happen in b_evac (ACT + GPSIMD).
                    ns = nrmp.tile([128, 32], f32, tag="nsg")
                    nc.vector.reciprocal(
                        ns[:, 0:21].rearrange("p (g i o) -> p g i o", g=3, o=1),
                        pa4[:, :, :, 64:65],
                    )
                    nc.vector.reciprocal(
                        ns[:, 21:28].rearrange("p (i o) -> p i o", o=1),
                        pb4[:, 0, 0:7, 64:65],
                    )
                    nc.vector.reciprocal(
                        ns[:, 28:32].rearrange("p (i o) -> p i o", o=1),
                        pb4[:, 1, 0:4, 64:65],
                    )
                    st["ns"] = ns
                    st["pa4"], st["pb4"] = pa4, pb4
                    # slice a psum stays live until b_evac's ACT copies; M2b
                    # uses the OTHER psum pool generation (bufs=2).
                else:
                    # slice b: direct DVE broadcast multiplies from psum
                    base = FREE
                    nsa = nrmp.tile([128, 21], f32, tag="nsa")
                    nsa3 = nsa.rearrange("p (g i) -> p g i", g=3)
                    nc.vector.reciprocal(
                        nsa.rearrange("p (g i o) -> p g i o", g=3, o=1),
                        pa4[:, :, :, 64:65],
                    )
                    nc.vector.tensor_tensor(
                        out_sb[:, base : base + 1344].rearrange(
                            "p (g i c) -> p g i c", g=3, c=64
                        ),
                        pa4[:, :, :, 0:64],
                        nsa3.broadcast_to([128, 3, 7, 64]),
                        Alu.mult,
                    )
                    nsb = nrmp.tile([128, 14], f32, tag="nsb")
                    nsb3 = nsb.rearrange("p (g i) -> p g i", g=2)
                    for bi, (g0, gs) in enumerate(GROUPS[3:]):
                        nc.vector.reciprocal(
                            nsb[:, bi * 7 : bi * 7 + gs].rearrange(
                                "p (i o) -> p i o", o=1
                            ),
                            pb4[:, bi, 0:gs, 64:65],
                        )
                    for bi, (g0, gs) in enumerate(GROUPS[3:]):
                        nc.vector.tensor_tensor(
                            out_sb[
                                :, base + g0 * 64 : base + (g0 + gs) * 64
                            ].rearrange("p (i c) -> p i c", c=64),
                            pb4[:, bi, 0:gs, 0:64],
                            nsb3[:, bi, 0:gs].broadcast_to([128, gs, 64]),
                            Alu.mult,
                        )

        def b_evac(pair, st):
            """Slice-a evac (ACT psum->sbuf copies + one GPSIMD broadcast
            multiply; GPSIMD has no PSUM port) + the pair's single output
            DMA, issued from the SCALAR HWDGE ring - critically NOT the
            sync ring, where its descriptors would sit behind the entire
            remaining INPUT stream (and not GPSIMD SWDGE, which moved the
            4.2 MB of outputs at only ~100 GB/s). Emitted two pairs behind
            fa so the ACT copies never FIFO-stall the pacing ek/eq ops."""
            out_sb, ns = st["out_sb"], st["ns"]
            pa4, pb4 = st["pa4"], st["pb4"]
            stage = stp.tile([128, FREE], bf16)
            nc.scalar.copy(
                stage[:, 0:1344].rearrange("p (g i c) -> p g i c", g=3, c=64),
                pa4[:, :, :, 0:64],
            )
            nc.scalar.copy(
                stage[:, 1344:1792].rearrange("p (i c) -> p i c", c=64),
                pb4[:, 0, 0:7, 0:64],
            )
            nc.scalar.copy(
                stage[:, 1792:2048].rearrange("p (i c) -> p i c", c=64),
                pb4[:, 1, 0:4, 0:64],
            )
            nc.gpsimd.tensor_tensor(
                out_sb[:, 0:FREE].rearrange("p (i c) -> p i c", c=64),
                stage.rearrange("p (i c) -> p i c", c=64),
                ns.rearrange("p i -> p i ()").broadcast_to([128, 32, 64]),
                Alu.mult,
            )
            nc.scalar.dma_start(out=outc[pair][:, :], in_=out_sb[:, :])

        # Software pipeline, staggered per engine so every FIFO always has
        # ready work ahead of any dependency-blocked instruction.
        st = {}
        st[0] = fa(0)
        fb(0, st[0])
        st[1] = fa(1)
        fb(1, st[1])
        m(0, st[0])
        b_pe(0, st[0])
        b_evac(0, st[0])
        st[2] = fa(2)
        fb(2, st[2])
        m(1, st[1])
        b_pe(1, st[1])
        b_evac(1, st[1])
        st[3] = fa(3)
        fb(3, st[3])
        m(2, st[2])
        b_pe(2, st[2])
        b_evac(2, st[2])
        m(3, st[3])
        b_pe(3, st[3])
        b_evac(3, st[3])

    nc.compile()
    return nc


def _get_program():
    if "p" not in _programs:
        _programs["p"] = _build_program()
    return _programs["p"]


def _pack_inputs(query, key, value, attention_mask):
    """Shard + lay out inputs for the 8 cores (all bf16, fused per pair)."""
    q4 = np.asarray(query, dtype=np.float32).reshape(B * H, S, D)
    k4 = np.asarray(key, dtype=np.float32).reshape(B * H, S, D)
    v4 = np.asarray(value, dtype=np.float32).reshape(B * H, S, D)
    am = np.asarray(attention_mask, dtype=np.float32)

    inp = np.empty((N_CORES, PAIRS, 128, INW), dtype=BF16)
    # kp: [g, n, d] -> [core, pair, p, t*128 + s*64 + d]
    k6 = k4.reshape(N_CORES, PAIRS, 2, NT, 128, D)
    inp[:, :, :, 0:KW] = (
        k6.transpose(0, 1, 4, 3, 2, 5).reshape(N_CORES, PAIRS, 128, KW).astype(BF16)
    )
    # vx: [V*mask | mask] -> [core, pair, p, KW + t*130 + s*65 + c]
    mrow = np.repeat(am, H, axis=0).reshape(B * H, S, 1)  # [g, n, 1]
    if np.all(am == 1.0):
        vext = np.concatenate([v4, mrow], axis=-1)
    else:
        vext = np.concatenate([v4 * mrow, mrow], axis=-1)
    v6 = vext.reshape(N_CORES, PAIRS, 2, NT, 128, D + 1)
    inp[:, :, :, KW : KW + VW] = (
        v6.transpose(0, 1, 4, 3, 2, 5).reshape(N_CORES, PAIRS, 128, VW).astype(BF16)
    )
    # qT: [g, d, n] -> [core, pair, 2*64 d, n], pre-scaled by 1/8 (exact)
    inp[:, :, :, KW + VW : INW] = (
        np.ascontiguousarray((q4 * 0.125).transpose(0, 2, 1))
        .reshape(N_CORES, PAIRS, 2 * D, S)
        .astype(BF16)
    )
    return [{"inp": inp[c]} for c in range(N_CORES)]


def _unpack_output(results):
    outs = np.stack([r["outc"] for r in results])  # [cores, PAIRS, 128, 2*FREE]
    outs = outs.astype(np.float32).reshape(N_CORES, PAIRS, 128, 2, NT, D)
    outs = outs.transpose(0, 1, 3, 4, 2, 5)  # [cores, pair, s, t, p, d]
    return np.ascontiguousarray(outs).reshape(B, H, S, D)


def kernel(query, key, value, attention_mask):
    from concourse.bass_utils import run_bass_kernel_spmd

    in_maps = _pack_inputs(query, key, value, attention_mask)
    nc = _get_program()
    res = run_bass_kernel_spmd(nc, in_maps, core_ids=list(range(N_CORES)))
    return _unpack_output(res.results)
